# revision 30
# baseline (speedup 1.0000x reference)
"""Luong attention kernel for Trainium2 (8 NeuronCores, data-parallel over batch).

Problem: B=64, S=2048, H=1024 fp32.
  dec_state = dec_state_h + dec_state_c                     (host)
  score[b,s] = sum_h enc[b,s,h] * dec_state[b,h]            (DVE fused mul+reduce)
  attn = softmax(score, axis=s)                             (PE transpose + DVE/ACT)
  context[b,h] = sum_s attn[b,s] * enc[b,s,h]               (PE matmul, bf16)

Each core handles 8 batches; enc slice is read from HBM exactly once.
"""
import sys

sys.path.insert(0, "/opt/trn_rl_repo")

import numpy as np

import concourse.bass as bass
import concourse.mybir as mybir
import concourse.tile as tile
from concourse import bacc

B_LOC = 8        # batches per core
S = 2048
H = 1024
T = S // 128     # 16 s-tiles of 128 per batch
HALVES = 2       # each batch's enc is DMA'd as two 4MB transfers
SO = T // HALVES  # 8 s-tiles per half

_CACHE = {}


def _build():
    f32 = mybir.dt.float32
    bf16 = mybir.dt.float16  # fp16: same PE rate as bf16, 8x better mantissa
    Copy = mybir.ActivationFunctionType.Copy
    Exp = mybir.ActivationFunctionType.Exp

    nc = bacc.Bacc(None, name="luong_attn", target_bir_lowering=False)
    enc_d = nc.dram_tensor("enc", (B_LOC, S, H), f32, kind="ExternalInput")
    dec_d = nc.dram_tensor("dec", (B_LOC, H), f32, kind="ExternalInput")
    ctx_d = nc.dram_tensor("ctx", (B_LOC, H), f32, kind="ExternalOutput")
    attn_d = nc.dram_tensor("attn", (B_LOC, S), f32, kind="ExternalOutput")
    ident_d = nc.inline_tensor(np.eye(128, dtype=np.float32), name="ident128")

    with tile.TileContext(nc) as tc:
        with (
            tc.tile_pool(name="E", bufs=3) as Epool,        # fp32 enc halves, 4MB each
            tc.tile_pool(name="Er", bufs=4) as Erpool,      # bf16 enc halves, 2MB each
            tc.tile_pool(name="dec", bufs=4) as decpool,
            tc.tile_pool(name="small", bufs=4) as small,
            tc.tile_pool(name="ctxout", bufs=3) as ctxout,
            tc.tile_pool(name="stage", bufs=1) as stage,
            tc.tile_pool(name="pctx", bufs=4, space="PSUM") as pctx,
            tc.tile_pool(name="psmall", bufs=2, space="PSUM") as psmall,
        ):
            attnT_all = stage.tile([T, B_LOC, 128], f32)
            dummy = stage.tile([128, 1], f32)
            ident = stage.tile([128, 128], f32)
            ones_row = stage.tile([1, 128], f32)
            nc.vector.memset(ones_row, 1.0)

            for b in range(B_LOC):
                # dec_state[b] broadcast across partitions, straight from DRAM
                dec_bc = decpool.tile([128, H], f32)
                src = bass.AP(
                    tensor=dec_d[b : b + 1, :].tensor,
                    offset=dec_d[b : b + 1, :].offset,
                    ap=[[0, 128], [1, H]],
                )
                nc.gpsimd.dma_start(out=dec_bc, in_=src)
                if b == 0:
                    nc.gpsimd.dma_start(out=ident, in_=ident_d[:, :])

                S_ = small.tile([128, T], f32)
                Ers = []
                for hh in range(HALVES):
                    # one 4MB DMA: s = (hh*SO + so)*128 + p
                    E = Epool.tile([128, SO, H], f32)
                    src = enc_d[b, hh * SO * 128 : (hh + 1) * SO * 128, :].rearrange(
                        "(so p) h -> p so h", p=128
                    )
                    if (b == 0 and hh == 0) or (b == B_LOC - 1 and hh == 1):
                        # fine-grained first load so scoring starts early
                        for qq in range(4):
                            nc.sync.dma_start(
                                out=E[:, qq * 2 : qq * 2 + 2, :],
                                in_=src[:, qq * 2 : qq * 2 + 2, :],
                            )
                    else:
                        nc.sync.dma_start(out=E, in_=src)
                    # bf16 copy for the context matmul (ACT), in 4 chunks
                    Er = Erpool.tile([128, SO, H], bf16)
                    for cc in range(4):
                        nc.scalar.copy(
                            out=Er[:, cc * 2 : cc * 2 + 2, :],
                            in_=E[:, cc * 2 : cc * 2 + 2, :],
                        )
                    Ers.append(Er)

                    # scores: one fused multiply+row-sum per s-tile (DVE)
                    for so in range(SO):
                        t = hh * SO + so
                        nc.vector.scalar_tensor_tensor(
                            out=dummy.broadcast_to([128, H]),
                            in0=E[:, so, :],
                            scalar=1.0,
                            in1=dec_bc,
                            op0=mybir.AluOpType.mult,
                            op1=mybir.AluOpType.mult,
                            accum_out=S_[:, t : t + 1],
                        )

                # ---- softmax over all 2048 scores of batch b ----
                m1 = small.tile([128, 1], f32)
                nc.vector.tensor_reduce(
                    out=m1, in_=S_, axis=mybir.AxisListType.X, op=mybir.AluOpType.max
                )
                pt = psmall.tile([1, 128], f32, tag="pp")
                nc.tensor.transpose(pt, m1, ident)
                ms = small.tile([1, 1], f32)
                nc.vector.tensor_reduce(
                    out=ms, in_=pt, axis=mybir.AxisListType.X, op=mybir.AluOpType.max
                )
                bc = psmall.tile([128, 1], f32, tag="pb")
                nc.tensor.matmul(bc, ones_row, ms, start=True, stop=True)
                negM = small.tile([128, 1], f32)
                nc.vector.tensor_scalar_mul(negM, bc, -1.0)

                W = small.tile([128, T], f32)
                r = small.tile([128, 1], f32)
                nc.scalar.activation(
                    out=W, in_=S_, func=Exp, bias=negM[:, 0:1], scale=1.0, accum_out=r
                )

                pr = psmall.tile([1, 128], f32, tag="pp")
                nc.tensor.transpose(pr, r, ident)
                Ls = small.tile([1, 1], f32)
                nc.vector.tensor_reduce(
                    out=Ls, in_=pr, axis=mybir.AxisListType.X, op=mybir.AluOpType.add
                )
                rLs = small.tile([1, 1], f32)
                nc.vector.reciprocal(rLs, Ls)
                bc2 = psmall.tile([128, 1], f32, tag="pb")
                nc.tensor.matmul(bc2, ones_row, rLs, start=True, stop=True)
                rL = small.tile([128, 1], f32)
                nc.vector.tensor_copy(out=rL, in_=bc2)

                # bf16 copy of the *unnormalized* weights for the matmul
                Wr = small.tile([128, T], bf16)
                nc.scalar.copy(out=Wr, in_=W)
                # normalized attention weights (fp32, exact path)
                Wn = small.tile([128, T], f32)
                nc.scalar.activation(
                    out=Wn, in_=W, func=Copy, scale=rL[:, 0:1]
                )
                # transpose weights so each partition holds a contiguous HBM run
                pw = psmall.tile([T, 128], f32, tag="pp")
                nc.tensor.transpose(pw, Wn, ident)
                nc.scalar.copy(out=attnT_all[:, b, :], in_=pw)

                # ---- context: PSUM-accumulated bf16 matmuls over all 16 s-tiles
                ctx_b = ctxout.tile([1, H], f32)
                for jh in range(2):  # output column half (N=512 each)
                    acc = pctx.tile([1, 512], f32)
                    k = 0
                    for hh in range(HALVES):
                        for so in range(SO):
                            t = hh * SO + so
                            nc.tensor.matmul(
                                acc,
                                Wr[:, t : t + 1],
                                Ers[hh][:, so, jh * 512 : (jh + 1) * 512],
                                start=(k == 0),
                                stop=(k == T - 1),
                            )
                            k += 1
                    nc.scalar.activation(
                        out=ctx_b[0:1, jh * 512 : (jh + 1) * 512], in_=acc,
                        func=Copy, scale=rLs[0:1, 0:1],
                    )
                nc.gpsimd.dma_start(
                    out=ctx_d[b : b + 1, :], in_=ctx_b
                )

            nc.gpsimd.dma_start(
                out=attn_d.rearrange("b (t p) -> t b p", p=128), in_=attnT_all
            )

    nc.compile()
    return nc


def _get_nc():
    if "nc" not in _CACHE:
        _CACHE["nc"] = _build()
    return _CACHE["nc"]


def kernel(dec_state_h, dec_state_c, enc_output, **_):
    from concourse.bass_utils import run_bass_kernel_spmd

    dec_state_h = np.asarray(dec_state_h, dtype=np.float32)
    dec_state_c = np.asarray(dec_state_c, dtype=np.float32)
    enc_output = np.ascontiguousarray(np.asarray(enc_output, dtype=np.float32))
    B = enc_output.shape[0]
    n_cores = B // B_LOC

    dec = dec_state_h + dec_state_c
    nc = _get_nc()

    in_maps = [
        {
            "enc": enc_output[c * B_LOC : (c + 1) * B_LOC],
            "dec": np.ascontiguousarray(dec[c * B_LOC : (c + 1) * B_LOC]),
        }
        for c in range(n_cores)
    ]
    res = run_bass_kernel_spmd(nc, in_maps, core_ids=list(range(n_cores)))
    ctx = np.concatenate([r["ctx"] for r in res.results], axis=0)
    attn = np.concatenate([r["attn"] for r in res.results], axis=0)
    return ctx.astype(np.float32), attn.reshape(B, S, 1).astype(np.float32)


# revision 35
# speedup vs baseline: 1.0269x; 1.0269x over previous
"""Luong attention kernel for Trainium2 (8 NeuronCores, data-parallel over batch).

Problem: B=64, S=2048, H=1024 fp32.
  dec_state = dec_state_h + dec_state_c                     (host)
  score[b,s] = sum_h enc[b,s,h] * dec_state[b,h]            (DVE fused mul+reduce)
  attn = softmax(score, axis=s)                             (PE transpose + DVE/ACT)
  context[b,h] = sum_s attn[b,s] * enc[b,s,h]               (PE matmul, bf16)

Each core handles 8 batches; enc slice is read from HBM exactly once.
"""
import sys

sys.path.insert(0, "/opt/trn_rl_repo")

import numpy as np

import concourse.bass as bass
import concourse.mybir as mybir
import concourse.tile as tile
from concourse import bacc

B_LOC = 8        # batches per core
S = 2048
H = 1024
T = S // 128     # 16 s-tiles of 128 per batch
HALVES = 2       # each batch's enc is DMA'd as two 4MB transfers
SO = T // HALVES  # 8 s-tiles per half

_CACHE = {}


def _build():
    f32 = mybir.dt.float32
    bf16 = mybir.dt.float16  # fp16: same PE rate as bf16, 8x better mantissa
    Copy = mybir.ActivationFunctionType.Copy
    Exp = mybir.ActivationFunctionType.Exp

    nc = bacc.Bacc(None, name="luong_attn", target_bir_lowering=False)
    enc_d = nc.dram_tensor("enc", (B_LOC, S, H), f32, kind="ExternalInput")
    dec_d = nc.dram_tensor("dec", (B_LOC, H), f32, kind="ExternalInput")
    ctx_d = nc.dram_tensor("ctx", (B_LOC, H), f32, kind="ExternalOutput")
    attn_d = nc.dram_tensor("attn", (B_LOC, S), f32, kind="ExternalOutput")
    ident_d = nc.inline_tensor(np.eye(128, dtype=np.float32), name="ident128")

    with tile.TileContext(nc) as tc:
        with (
            tc.tile_pool(name="E", bufs=3) as Epool,        # fp32 enc halves, 4MB each
            tc.tile_pool(name="Er", bufs=4) as Erpool,      # bf16 enc halves, 2MB each
            tc.tile_pool(name="dec", bufs=4) as decpool,
            tc.tile_pool(name="small", bufs=4) as small,
            tc.tile_pool(name="ctxout", bufs=3) as ctxout,
            tc.tile_pool(name="stage", bufs=1) as stage,
            tc.tile_pool(name="pctx", bufs=4, space="PSUM") as pctx,
            tc.tile_pool(name="psmall", bufs=2, space="PSUM") as psmall,
        ):
            attnT_all = stage.tile([T, B_LOC, 128], f32)
            dummy = stage.tile([128, 1], f32)
            ident = stage.tile([128, 128], f32)
            ones_row = stage.tile([1, 128], f32)
            nc.vector.memset(ones_row, 1.0)

            for b in range(B_LOC):
                # dec_state[b] broadcast across partitions, straight from DRAM
                dec_bc = decpool.tile([128, H], f32)
                src = bass.AP(
                    tensor=dec_d[b : b + 1, :].tensor,
                    offset=dec_d[b : b + 1, :].offset,
                    ap=[[0, 128], [1, H]],
                )
                nc.gpsimd.dma_start(out=dec_bc, in_=src)
                if b == 0:
                    nc.gpsimd.dma_start(out=ident, in_=ident_d[:, :])

                S_ = small.tile([128, T], f32)
                Ers = []
                for hh in range(HALVES):
                    # one 4MB DMA: s = (hh*SO + so)*128 + p
                    E = Epool.tile([128, SO, H], f32)
                    src = enc_d[b, hh * SO * 128 : (hh + 1) * SO * 128, :].rearrange(
                        "(so p) h -> p so h", p=128
                    )
                    if (b == 0 and hh == 0) or (b >= B_LOC - 3):
                        # fine-grained first load so scoring starts early
                        for qq in range(4):
                            nc.sync.dma_start(
                                out=E[:, qq * 2 : qq * 2 + 2, :],
                                in_=src[:, qq * 2 : qq * 2 + 2, :],
                            )
                    else:
                        nc.sync.dma_start(out=E, in_=src)
                    # bf16 copy for the context matmul (ACT), in 4 chunks
                    Er = Erpool.tile([128, SO, H], bf16)
                    for cc in range(4):
                        nc.scalar.copy(
                            out=Er[:, cc * 2 : cc * 2 + 2, :],
                            in_=E[:, cc * 2 : cc * 2 + 2, :],
                        )
                    Ers.append(Er)

                    # scores: one fused multiply+row-sum per s-tile (DVE)
                    for so in range(SO):
                        t = hh * SO + so
                        nc.vector.scalar_tensor_tensor(
                            out=dummy.broadcast_to([128, H]),
                            in0=E[:, so, :],
                            scalar=1.0,
                            in1=dec_bc,
                            op0=mybir.AluOpType.mult,
                            op1=mybir.AluOpType.mult,
                            accum_out=S_[:, t : t + 1],
                        )

                # ---- softmax over all 2048 scores of batch b ----
                m1 = small.tile([128, 1], f32)
                nc.vector.tensor_reduce(
                    out=m1, in_=S_, axis=mybir.AxisListType.X, op=mybir.AluOpType.max
                )
                pt = psmall.tile([1, 128], f32, tag="pp")
                nc.tensor.transpose(pt, m1, ident)
                ms = small.tile([1, 1], f32)
                nc.vector.tensor_reduce(
                    out=ms, in_=pt, axis=mybir.AxisListType.X, op=mybir.AluOpType.max
                )
                bc = psmall.tile([128, 1], f32, tag="pb")
                nc.tensor.matmul(bc, ones_row, ms, start=True, stop=True)
                negM = small.tile([128, 1], f32)
                nc.vector.tensor_scalar_mul(negM, bc, -1.0)

                W = small.tile([128, T], f32)
                r = small.tile([128, 1], f32)
                nc.scalar.activation(
                    out=W, in_=S_, func=Exp, bias=negM[:, 0:1], scale=1.0, accum_out=r
                )

                pr = psmall.tile([1, 128], f32, tag="pp")
                nc.tensor.transpose(pr, r, ident)
                Ls = small.tile([1, 1], f32)
                nc.vector.tensor_reduce(
                    out=Ls, in_=pr, axis=mybir.AxisListType.X, op=mybir.AluOpType.add
                )
                rLs = small.tile([1, 1], f32)
                nc.vector.reciprocal(rLs, Ls)
                bc2 = psmall.tile([128, 1], f32, tag="pb")
                nc.tensor.matmul(bc2, ones_row, rLs, start=True, stop=True)
                rL = small.tile([128, 1], f32)
                nc.vector.tensor_copy(out=rL, in_=bc2)

                # bf16 copy of the *unnormalized* weights for the matmul
                Wr = small.tile([128, T], bf16)
                nc.scalar.copy(out=Wr, in_=W)
                # normalized attention weights (fp32, exact path)
                Wn = small.tile([128, T], f32)
                nc.scalar.activation(
                    out=Wn, in_=W, func=Copy, scale=rL[:, 0:1]
                )
                # transpose weights so each partition holds a contiguous HBM run
                pw = psmall.tile([T, 128], f32, tag="pp")
                nc.tensor.transpose(pw, Wn, ident)
                nc.scalar.copy(out=attnT_all[:, b, :], in_=pw)

                # ---- context: PSUM-accumulated bf16 matmuls over all 16 s-tiles
                ctx_b = ctxout.tile([1, H], f32)
                for jh in range(2):  # output column half (N=512 each)
                    acc = pctx.tile([1, 512], f32)
                    k = 0
                    for hh in range(HALVES):
                        for so in range(SO):
                            t = hh * SO + so
                            nc.tensor.matmul(
                                acc,
                                Wr[:, t : t + 1],
                                Ers[hh][:, so, jh * 512 : (jh + 1) * 512],
                                start=(k == 0),
                                stop=(k == T - 1),
                            )
                            k += 1
                    nc.scalar.activation(
                        out=ctx_b[0:1, jh * 512 : (jh + 1) * 512], in_=acc,
                        func=Copy, scale=rLs[0:1, 0:1],
                    )
                nc.gpsimd.dma_start(
                    out=ctx_d[b : b + 1, :], in_=ctx_b
                )

            nc.gpsimd.dma_start(
                out=attn_d.rearrange("b (t p) -> t b p", p=128), in_=attnT_all
            )

    nc.compile()
    return nc


def _get_nc():
    if "nc" not in _CACHE:
        _CACHE["nc"] = _build()
    return _CACHE["nc"]


def kernel(dec_state_h, dec_state_c, enc_output, **_):
    from concourse.bass_utils import run_bass_kernel_spmd

    dec_state_h = np.asarray(dec_state_h, dtype=np.float32)
    dec_state_c = np.asarray(dec_state_c, dtype=np.float32)
    enc_output = np.ascontiguousarray(np.asarray(enc_output, dtype=np.float32))
    B = enc_output.shape[0]
    n_cores = B // B_LOC

    dec = dec_state_h + dec_state_c
    nc = _get_nc()

    in_maps = [
        {
            "enc": enc_output[c * B_LOC : (c + 1) * B_LOC],
            "dec": np.ascontiguousarray(dec[c * B_LOC : (c + 1) * B_LOC]),
        }
        for c in range(n_cores)
    ]
    res = run_bass_kernel_spmd(nc, in_maps, core_ids=list(range(n_cores)))
    ctx = np.concatenate([r["ctx"] for r in res.results], axis=0)
    attn = np.concatenate([r["attn"] for r in res.results], axis=0)
    return ctx.astype(np.float32), attn.reshape(B, S, 1).astype(np.float32)


# revision 39
# speedup vs baseline: 1.0293x; 1.0023x over previous
"""Luong attention kernel for Trainium2 (8 NeuronCores, data-parallel over batch).

Problem: B=64, S=2048, H=1024 fp32.
  dec_state = dec_state_h + dec_state_c                     (host)
  score[b,s] = sum_h enc[b,s,h] * dec_state[b,h]            (DVE fused mul+reduce)
  attn = softmax(score, axis=s)                             (PE transpose + DVE/ACT)
  context[b,h] = sum_s attn[b,s] * enc[b,s,h]               (PE matmul, bf16)

Each core handles 8 batches; enc slice is read from HBM exactly once.
"""
import sys

sys.path.insert(0, "/opt/trn_rl_repo")

import numpy as np

import concourse.bass as bass
import concourse.mybir as mybir
import concourse.tile as tile
from concourse import bacc

B_LOC = 8        # batches per core
S = 2048
H = 1024
T = S // 128     # 16 s-tiles of 128 per batch
HALVES = 2       # each batch's enc is DMA'd as two 4MB transfers
SO = T // HALVES  # 8 s-tiles per half

_CACHE = {}


def _build():
    f32 = mybir.dt.float32
    bf16 = mybir.dt.float16  # fp16: same PE rate as bf16, 8x better mantissa
    Copy = mybir.ActivationFunctionType.Copy
    Exp = mybir.ActivationFunctionType.Exp

    nc = bacc.Bacc(None, name="luong_attn", target_bir_lowering=False)
    enc_d = nc.dram_tensor("enc", (B_LOC, S, H), f32, kind="ExternalInput")
    dec_d = nc.dram_tensor("dec", (B_LOC, H), f32, kind="ExternalInput")
    ctx_d = nc.dram_tensor("ctx", (B_LOC, H), f32, kind="ExternalOutput")
    attn_d = nc.dram_tensor("attn", (B_LOC, S), f32, kind="ExternalOutput")
    ident_d = nc.inline_tensor(np.eye(128, dtype=np.float32), name="ident128")

    with tile.TileContext(nc) as tc:
        with (
            tc.tile_pool(name="E", bufs=3) as Epool,        # fp32 enc halves, 4MB each
            tc.tile_pool(name="Er", bufs=4) as Erpool,      # bf16 enc halves, 2MB each
            tc.tile_pool(name="dec", bufs=4) as decpool,
            tc.tile_pool(name="small", bufs=4) as small,
            tc.tile_pool(name="ctxout", bufs=3) as ctxout,
            tc.tile_pool(name="stage", bufs=1) as stage,
            tc.tile_pool(name="pctx", bufs=4, space="PSUM") as pctx,
            tc.tile_pool(name="psmall", bufs=2, space="PSUM") as psmall,
        ):
            attnT_all = stage.tile([T, B_LOC, 128], f32)
            dummy = stage.tile([128, 1], f32)
            ident = stage.tile([128, 128], f32)
            ones_row = stage.tile([1, 128], f32)
            nc.vector.memset(ones_row, 1.0)

            for b in range(B_LOC):
                # dec_state[b] broadcast across partitions, straight from DRAM
                dec_bc = decpool.tile([128, H], f32)
                src = bass.AP(
                    tensor=dec_d[b : b + 1, :].tensor,
                    offset=dec_d[b : b + 1, :].offset,
                    ap=[[0, 128], [1, H]],
                )
                nc.gpsimd.dma_start(out=dec_bc, in_=src)
                if b == 0:
                    nc.gpsimd.dma_start(out=ident, in_=ident_d[:, :])

                S_ = small.tile([128, T], f32)
                Ers = []
                for hh in range(HALVES):
                    # one 4MB DMA: s = (hh*SO + so)*128 + p
                    E = Epool.tile([128, SO, H], f32)
                    src = enc_d[b, hh * SO * 128 : (hh + 1) * SO * 128, :].rearrange(
                        "(so p) h -> p so h", p=128
                    )
                    if (b == 0 and hh == 0) or (b >= B_LOC - 3):
                        # fine-grained first load so scoring starts early
                        for qq in range(4):
                            nc.sync.dma_start(
                                out=E[:, qq * 2 : qq * 2 + 2, :],
                                in_=src[:, qq * 2 : qq * 2 + 2, :],
                            )
                    else:
                        nc.sync.dma_start(out=E, in_=src)
                    # bf16 copy for the context matmul (ACT), in 4 chunks
                    Er = Erpool.tile([128, SO, H], bf16)
                    for cc in range(4):
                        nc.scalar.copy(
                            out=Er[:, cc * 2 : cc * 2 + 2, :],
                            in_=E[:, cc * 2 : cc * 2 + 2, :],
                        )
                    Ers.append(Er)

                    # scores: one fused multiply+row-sum per s-tile (DVE)
                    for so in range(SO):
                        t = hh * SO + so
                        nc.vector.scalar_tensor_tensor(
                            out=dummy.broadcast_to([128, H]),
                            in0=E[:, so, :],
                            scalar=1.0,
                            in1=dec_bc,
                            op0=mybir.AluOpType.mult,
                            op1=mybir.AluOpType.mult,
                            accum_out=S_[:, t : t + 1],
                        )

                # ---- softmax over all 2048 scores of batch b ----
                m1 = small.tile([128, 1], f32)
                nc.vector.tensor_reduce(
                    out=m1, in_=S_, axis=mybir.AxisListType.X, op=mybir.AluOpType.max
                )
                pt = psmall.tile([1, 128], f32, tag="pp")
                nc.tensor.transpose(pt, m1, ident)
                ms = small.tile([1, 1], f32)
                nc.vector.tensor_reduce(
                    out=ms, in_=pt, axis=mybir.AxisListType.X, op=mybir.AluOpType.max
                )
                bc = psmall.tile([128, 1], f32, tag="pb")
                nc.tensor.matmul(bc, ones_row, ms, start=True, stop=True)
                negM = small.tile([128, 1], f32)
                nc.vector.tensor_scalar_mul(negM, bc, -1.0)

                W = small.tile([128, T], f32)
                r = small.tile([128, 1], f32)
                nc.scalar.activation(
                    out=W, in_=S_, func=Exp, bias=negM[:, 0:1], scale=1.0, accum_out=r
                )

                pr = psmall.tile([1, 128], f32, tag="pp")
                nc.tensor.transpose(pr, r, ident)
                Ls = small.tile([1, 1], f32)
                nc.vector.tensor_reduce(
                    out=Ls, in_=pr, axis=mybir.AxisListType.X, op=mybir.AluOpType.add
                )
                rLs = small.tile([1, 1], f32)
                nc.vector.reciprocal(rLs, Ls)
                bc2 = psmall.tile([128, 1], f32, tag="pb")
                nc.tensor.matmul(bc2, ones_row, rLs, start=True, stop=True)
                rL = small.tile([128, 1], f32)
                nc.vector.tensor_copy(out=rL, in_=bc2)

                # bf16 copy of the *unnormalized* weights for the matmul
                Wr = small.tile([128, T], bf16)
                nc.scalar.copy(out=Wr, in_=W)
                # normalized attention weights (fp32, exact path)
                Wn = small.tile([128, T], f32)
                nc.scalar.activation(
                    out=Wn, in_=W, func=Copy, scale=rL[:, 0:1]
                )
                # transpose weights so each partition holds a contiguous HBM run
                pw = psmall.tile([T, 128], f32, tag="pp")
                nc.tensor.transpose(pw, Wn, ident)
                nc.scalar.copy(out=attnT_all[:, b, :], in_=pw)

                # ---- context: PSUM-accumulated bf16 matmuls over all 16 s-tiles
                ctx_b = ctxout.tile([1, H], f32)
                for jh in range(2):  # output column half (N=512 each)
                    acc = pctx.tile([1, 512], f32)
                    k = 0
                    for hh in range(HALVES):
                        for so in range(SO):
                            t = hh * SO + so
                            nc.tensor.matmul(
                                acc,
                                Wr[:, t : t + 1],
                                Ers[hh][:, so, jh * 512 : (jh + 1) * 512],
                                start=(k == 0),
                                stop=(k == T - 1),
                            )
                            k += 1
                    nc.scalar.activation(
                        out=ctx_b[0:1, jh * 512 : (jh + 1) * 512], in_=acc,
                        func=Copy, scale=rLs[0:1, 0:1],
                    )
                eng = nc.sync if b == B_LOC - 1 else nc.gpsimd
                eng.dma_start(
                    out=ctx_d[b : b + 1, :], in_=ctx_b
                )

            nc.sync.dma_start(
                out=attn_d.rearrange("b (t p) -> t b p", p=128), in_=attnT_all
            )

    nc.compile()
    return nc


def _get_nc():
    if "nc" not in _CACHE:
        _CACHE["nc"] = _build()
    return _CACHE["nc"]


def kernel(dec_state_h, dec_state_c, enc_output, **_):
    from concourse.bass_utils import run_bass_kernel_spmd

    dec_state_h = np.asarray(dec_state_h, dtype=np.float32)
    dec_state_c = np.asarray(dec_state_c, dtype=np.float32)
    enc_output = np.ascontiguousarray(np.asarray(enc_output, dtype=np.float32))
    B = enc_output.shape[0]
    n_cores = B // B_LOC

    dec = dec_state_h + dec_state_c
    nc = _get_nc()

    in_maps = [
        {
            "enc": enc_output[c * B_LOC : (c + 1) * B_LOC],
            "dec": np.ascontiguousarray(dec[c * B_LOC : (c + 1) * B_LOC]),
        }
        for c in range(n_cores)
    ]
    res = run_bass_kernel_spmd(nc, in_maps, core_ids=list(range(n_cores)))
    ctx = np.concatenate([r["ctx"] for r in res.results], axis=0)
    attn = np.concatenate([r["attn"] for r in res.results], axis=0)
    return ctx.astype(np.float32), attn.reshape(B, S, 1).astype(np.float32)
